# revision 2
# baseline (speedup 1.0000x reference)
"""TRN2 Bass kernel for nn_GAT (gnn_message_passing).

3-layer GAT stack: per layer h = relu(x@W+b); e = lrelu(s1[i]+s2[j]) masked by
adj; x += softmax_j(e) @ h.   B=8 graphs, N=2048 nodes, D=128 features.

Sharding: data-parallel over the batch dim - one graph per NeuronCore (8
cores), tiny per-layer weights replicated to every core.

Device algorithm (per core, features on partitions, node index on free axis):
  exp(lrelu(t)) with t = s1[i] + s2[j] factors through
    exp(lrelu(t)) = exp(0.2 t) * max(exp(0.8 t), 1)
  and exp(0.8 t) = exp(0.8 s1[i]) * exp(0.8 s2[j]) is RANK-1, so after the
  softmax cancellation of exp(0.2 s1[i]) the numerator weight is
    pp[j,i] = mask[j,i] * max(u[i] * w[j], e2[j])
  with u = exp(0.8 s1), w = exp(s2), e2 = exp(0.2 s2)  (e2 folded into the
  max so den = sum_j pp needs only a constant all-ones stationary).
  Per 128x2048 j-tile the NxN work is ONE DVE tensor_scalar (4x mode:
  (ubc*w_j) max e2_j) plus ONE fp16 mask multiply (DVE 2x / GPSIMD) - the
  N^2 ACT exp of the naive form disappears entirely (exp runs only on the
  N-sized s1/s2 vectors).
  yT[d,i] = sum_j relu(h)[j,d] pp[j,i]  and  den[i] = sum_j pp[j,i]
  accumulate on the PE in fp32 PSUM; 1/den = exp(-ln den) on ACT;
  xT += yT * (1/den).

Host side: x -> xT fp16, adj -> maskT fp16, weights pre-cast fp16; outputs
come back transposed fp32 and are flipped on the host.
"""

import numpy as np

B, N, D, L = 8, 2048, 128, 3
NT = N // 128
NCH = N // 512

# j-tiles whose mask-multiply runs on GPSIMD instead of DVE (load balance)
GPS_TILES = (3, 8, 13)


def _apply_tilefix():
    """This walrus build rejects >1 sync wait on an instruction; the stock
    Tile exit-drain carries several. Spread them across single-wait NOPs."""
    import concourse.tile as tile_mod
    from concourse import mybir

    def _patched_drain_and_barrier(self, tick_clock, wait_clock):
        from concourse.tile import ScopedClock

        drain_inst = self.nc.sync.drain()
        wait_clock.add_sem_waits(
            drain_inst.ins, ScopedClock({None: tick_clock.global_clock})
        )
        si = drain_inst.ins.sync_info
        if si is not None and len(si.on_wait) > 1:
            extra = list(si.on_wait[1:])
            del si.on_wait[1:]
            for w in extra:
                nop = self.nc.sync.nop()
                nop.ins.sync_info = mybir.SyncInfo(on_wait=[w], on_update=[])
        self.nc.all_engine_barrier()
        assert self.sems is not None
        popped = self.nc._tile_sem_poison_stack.pop()
        assert popped is self._sem_poison
        self.nc.clear_and_free_semaphores(list(self.sems.allocated().values()))
        self.nc.all_engine_barrier()

    tile_mod.TileContext._drain_and_barrier = _patched_drain_and_barrier


def _split_sync_waits(nc):
    """Hoist extra sync waits onto same-engine NOPs (walrus one-wait limit)."""
    from concourse import mybir

    n = 0
    for fn in nc.m.functions:
        for bb in fn.blocks:
            new_insts = []
            changed = False
            for inst in bb.instructions:
                si = inst.sync_info
                if si is not None and len(si.on_wait) > 1:
                    extra = list(si.on_wait[1:])
                    del si.on_wait[1:]
                    for w in extra:
                        nop = mybir.InstNoOp(name=f"waitsplit-{n}", ins=[], outs=[])
                        n += 1
                        nop.engine = inst.engine
                        nop.sync_info = mybir.SyncInfo(on_wait=[w], on_update=[])
                        new_insts.append(nop)
                    changed = True
                new_insts.append(inst)
            if changed:
                bb.instructions[:] = new_insts
    return n


def build_gat(reps=1):
    """Build the Bass program. reps>1 wraps the body in a For_i (timing)."""
    import contextlib

    import concourse.bass as bass
    import concourse.tile as tile
    from concourse import mybir

    f32 = mybir.dt.float32
    f16 = mybir.dt.float16
    A = mybir.AluOpType
    F = mybir.ActivationFunctionType

    nc = bass.Bass("TRN2", target_bir_lowering=False, debug=False, num_devices=8)

    xt_in = nc.dram_tensor("xt_in", [128, N], f16, kind="ExternalInput").ap()
    maskt = nc.dram_tensor("maskt", [N, N], f16, kind="ExternalInput").ap()
    wg16 = nc.dram_tensor("wg16", [L, D, D], f16, kind="ExternalInput").ap()
    bgc = nc.dram_tensor("bgc", [L, D], f32, kind="ExternalInput").ap()
    bgr16 = nc.dram_tensor("bgr16", [L, D], f16, kind="ExternalInput").ap()
    aa16 = nc.dram_tensor("aa16", [L, 2 * D], f16, kind="ExternalInput").ap()
    xt_out = nc.dram_tensor("xt_out", [128, N], f32, kind="ExternalOutput").ap()

    with tile.TileContext(nc) as tc:
        ctx = contextlib.ExitStack()
        with ctx:
            consts = ctx.enter_context(tc.tile_pool(name="consts", bufs=1))
            mask_pool = ctx.enter_context(tc.tile_pool(name="mask", bufs=1))
            xt_pool = ctx.enter_context(tc.tile_pool(name="xt", bufs=2))
            hT_pool = ctx.enter_context(tc.tile_pool(name="hT", bufs=1))
            hh_pool = ctx.enter_context(tc.tile_pool(name="hh", bufs=1))
            ubc_pool = ctx.enter_context(tc.tile_pool(name="ubc", bufs=1))
            vec_pool = ctx.enter_context(tc.tile_pool(name="vec", bufs=1))
            att_pool = ctx.enter_context(tc.tile_pool(name="att", bufs=3))
            norm_pool = ctx.enter_context(tc.tile_pool(name="norm", bufs=1))

            ones_row = consts.tile([1, 128], f16)
            nc.vector.memset(ones_row, 1.0)
            ones128 = consts.tile([128, 128], f16)
            nc.vector.memset(ones128, 1.0)
            Ws, b_cols, a1cols, a2cols, b_rows = [], [], [], [], []
            for l in range(L):
                W = consts.tile([128, 128], f16, tag=f"W{l}")
                nc.gpsimd.dma_start(out=W[:], in_=wg16[l])
                Ws.append(W)
                bc = consts.tile([128, 1], f32, tag=f"bc{l}")
                nc.gpsimd.dma_start(
                    out=bc[:], in_=bgc[l].rearrange("(d one) -> d one", one=1)
                )
                b_cols.append(bc)
                a1c = consts.tile([128, 1], f16, tag=f"a1{l}")
                nc.gpsimd.dma_start(
                    out=a1c[:], in_=aa16[l, 0:D].rearrange("(d one) -> d one", one=1)
                )
                a1cols.append(a1c)
                a2c = consts.tile([128, 1], f16, tag=f"a2{l}")
                nc.gpsimd.dma_start(
                    out=a2c[:],
                    in_=aa16[l, D : 2 * D].rearrange("(d one) -> d one", one=1),
                )
                a2cols.append(a2c)
                br = consts.tile([1, 128], f16, tag=f"br{l}")
                nc.gpsimd.dma_start(
                    out=br[:], in_=bgr16[l].rearrange("(one d) -> one d", one=1)
                )
                b_rows.append(br)

            def body():
                mask_sb = mask_pool.tile([128, NT * N], f16)
                x16 = xt_pool.tile([128, N], f16)
                nc.sync.dma_start(out=x16[:], in_=xt_in[:])
                for jt in range(NT):
                    nc.sync.dma_start(
                        out=mask_sb[:, jt * N : (jt + 1) * N],
                        in_=maskt[jt * 128 : (jt + 1) * 128, :],
                    )

                for l in range(L):
                    W = Ws[l]
                    prep_ctx = contextlib.ExitStack()
                    ps_big = prep_ctx.enter_context(
                        tc.tile_pool(name=f"ps_big{l}", bufs=1, space="PSUM")
                    )
                    ps_small = prep_ctx.enter_context(
                        tc.tile_pool(name=f"ps_small{l}", bufs=2, space="PSUM")
                    )
                    ps_col = prep_ctx.enter_context(
                        tc.tile_pool(name=f"ps_col{l}", bufs=1, space="PSUM")
                    )
                    # hT = relu(W.T @ xT + b)   [feat, node] fp16
                    hT_ps = ps_big.tile([128, N], f32, tag="big")
                    for c in range(NCH):
                        sl = slice(c * 512, (c + 1) * 512)
                        nc.tensor.matmul(hT_ps[:, sl], W[:], x16[:, sl])
                    hT = hT_pool.tile([128, N], f16)
                    nc.scalar.activation(
                        hT[:], hT_ps[:], F.Relu, bias=b_cols[l][:], scale=1.0
                    )
                    # s2 as columns -> w = exp(s2), e2 = exp(0.2 s2)
                    scols_ps = ps_col.tile([128, NT], f32, tag="scols")
                    for jt in range(NT):
                        nc.tensor.matmul(
                            scols_ps[:, jt : jt + 1],
                            hT[:, jt * 128 : (jt + 1) * 128],
                            a2cols[l][:],
                        )
                    wcols = vec_pool.tile([128, NT], f32, tag="w_sb")
                    nc.scalar.activation(wcols[:], scols_ps[:], F.Exp, scale=1.0)
                    e2cols = vec_pool.tile([128, NT], f32, tag="e2_sb")
                    nc.scalar.activation(e2cols[:], scols_ps[:], F.Exp, scale=0.2)
                    # s1 row (fp16, copied out of PSUM per 512-chunk on DVE)
                    s1row = vec_pool.tile([1, N], f16, tag="s1row")
                    for c in range(NCH):
                        sl = slice(c * 512, (c + 1) * 512)
                        s1_ps = ps_small.tile([1, 512], f32, tag="small")
                        nc.tensor.matmul(s1_ps[:], a1cols[l][:], hT[:, sl])
                        nc.vector.tensor_scalar(
                            s1row[:, sl], s1_ps[:], 0.0, None, A.add
                        )
                    # ubc = exp(0.8 * s1) broadcast to 128 partitions (exp is
                    # applied during the PSUM->SBUF copy on ACT)
                    ubc_ps = ps_big.tile([128, N], f32, tag="big")
                    for c in range(NCH):
                        sl = slice(c * 512, (c + 1) * 512)
                        nc.tensor.matmul(ubc_ps[:, sl], ones_row[:], s1row[:, sl])
                    ubc = ubc_pool.tile([128, N], f16)
                    nc.scalar.activation(ubc[:], ubc_ps[:], F.Exp, scale=0.8)
                    # hh = relu(h) node-major: recompute h per j-tile into one
                    # big PSUM strip, single DVE relu-copy out
                    hpp_ps = ps_big.tile([128, N], f32, tag="big")
                    for jt in range(NT):
                        sl = slice(jt * 128, (jt + 1) * 128)
                        nc.tensor.matmul(
                            hpp_ps[:, sl], x16[:, sl], W[:], start=True, stop=False
                        )
                        nc.tensor.matmul(
                            hpp_ps[:, sl], ones_row[:], b_rows[l][:],
                            start=False, stop=True,
                        )
                    hh = hh_pool.tile([128, NT * 128], f16, tag="hh")
                    nc.vector.tensor_scalar(hh[:], hpp_ps[:], 0.0, None, A.max)
                    prep_ctx.close()
                    # attention: pp = mask * max(ubc*w_j, e2_j)
                    attn_ctx = contextlib.ExitStack()
                    ps_y = attn_ctx.enter_context(
                        tc.tile_pool(name=f"ps_y{l}", bufs=1, space="PSUM")
                    )
                    ps_d = attn_ctx.enter_context(
                        tc.tile_pool(name=f"ps_d{l}", bufs=1, space="PSUM")
                    )
                    yT_ps = ps_y.tile([128, N], f32, tag="y")
                    den_ps = ps_d.tile([128, N], f32, tag="d")
                    for jt in range(NT):
                        q = att_pool.tile([128, N], f16, tag="q")
                        nc.vector.tensor_scalar(
                            q[:], ubc[:], wcols[:, jt : jt + 1],
                            e2cols[:, jt : jt + 1], A.mult, A.max,
                        )
                        pp = att_pool.tile([128, N], f16, tag="pp")
                        eng = nc.gpsimd if jt in GPS_TILES else nc.vector
                        eng.tensor_tensor(
                            pp[:], q[:], mask_sb[:, jt * N : (jt + 1) * N], A.mult
                        )
                        hsl = slice(jt * 128, (jt + 1) * 128)
                        for c in range(NCH):
                            sl = slice(c * 512, (c + 1) * 512)
                            nc.tensor.matmul(
                                yT_ps[:, sl], hh[:, hsl], pp[:, sl],
                                start=(jt == 0), stop=(jt == NT - 1),
                            )
                        for c in range(NCH):
                            sl = slice(c * 512, (c + 1) * 512)
                            nc.tensor.matmul(
                                den_ps[:, sl], ones128[:], pp[:, sl],
                                start=(jt == 0), stop=(jt == NT - 1),
                            )
                    # r = 1/den = exp(-ln den); xT_new = xT + yT * r
                    lnd = norm_pool.tile([128, N], f32, tag="nA")
                    nc.scalar.activation(lnd[:], den_ps[:], F.Ln)
                    r16 = norm_pool.tile([128, N], f16, tag="nB")
                    nc.scalar.activation(r16[:], lnd[:], F.Exp, scale=-1.0)
                    y16 = norm_pool.tile([128, N], f16, tag="nC")
                    nc.vector.tensor_scalar(y16[:], yT_ps[:], 0.0, None, A.add)
                    ytmp = norm_pool.tile([128, N], f16, tag="nD")
                    nc.vector.tensor_tensor(ytmp[:], y16[:], r16[:], A.mult)
                    attn_ctx.close()
                    if l < L - 1:
                        x_new = xt_pool.tile([128, N], f16)
                        nc.vector.tensor_tensor(x_new[:], ytmp[:], x16[:], A.add)
                        x16 = x_new
                    else:
                        x_out = norm_pool.tile([128, N], f32, tag="xout")
                        nc.vector.tensor_tensor(x_out[:], ytmp[:], x16[:], A.add)
                        nc.sync.dma_start(out=xt_out[:], in_=x_out[:])

            if reps == 1:
                body()
            else:
                with tc.For_i(0, reps, 1):
                    body()

    return nc


def host_prep(x, adj, Wg, bg, attn_a):
    in_maps = []
    for b in range(B):
        in_maps.append(
            {
                "xt_in": np.ascontiguousarray(x[b].T).astype(np.float16),
                "maskt": np.ascontiguousarray(adj[b].T > 0).astype(np.float16),
                "wg16": np.ascontiguousarray(Wg).astype(np.float16),
                "bgc": np.ascontiguousarray(bg, np.float32),
                "bgr16": np.ascontiguousarray(bg).astype(np.float16),
                "aa16": np.ascontiguousarray(attn_a).astype(np.float16),
            }
        )
    return in_maps


def host_post(results):
    return np.stack([results[b]["xt_out"].T for b in range(B)]).astype(np.float32)


def kernel(x, adj, Wg, bg, attn_a):
    x = np.asarray(x)
    adj = np.asarray(adj)
    Wg = np.asarray(Wg)
    bg = np.asarray(bg)
    attn_a = np.asarray(attn_a)

    _apply_tilefix()
    from concourse.bass_utils import run_bass_kernel_spmd

    nc = build_gat(reps=1)
    _split_sync_waits(nc)
    in_maps = host_prep(x, adj, Wg, bg, attn_a)
    res = run_bass_kernel_spmd(nc, in_maps, core_ids=list(range(B)))
    return host_post(res.results)


# revision 3
# speedup vs baseline: 1.1687x; 1.1687x over previous
"""TRN2 Bass kernel for nn_GAT (gnn_message_passing).  BASELINE (311453 ns).

3-layer GAT stack: per layer h = relu(x@W+b); e = lrelu(s1[i]+s2[j]) masked by
adj; x += softmax_j(e) @ h.   B=8 graphs, N=2048 nodes, D=128 features.

Sharding: data-parallel over the batch dim — one graph per NeuronCore (8
cores), tiny per-layer weights replicated to every core.

Device algorithm (per core, all layouts transposed: features on partitions,
node index on the free axis):
  lrelu(t) = 0.2*t + 0.8*relu(t), so with t = s1[i] + s2[j]:
    exp(lrelu(t)) = exp(0.2 s1[i]) * exp(0.2 s2[j]) * exp(0.8 relu(t))
  - exp(0.2 s1[i]) cancels between softmax numerator and denominator (no-max
    softmax is numerically safe here: |t| < 5, checked against the module)
  - exp(0.2 s2[j]) folds into the matmul stationaries (h''[j]=relu(h)[j]*e2[j],
    E2MAT[j]=e2[j])
  - per 128x2048 tile the NxN work is one fused DVE tensor_scalar
    (relu(s1bc + s2col)), one ACT Exp, one fp16 mask multiply
  yT[d,i] = sum_j h''[j,d] pp[j,i] and the replicated denominator
  den[i] = sum_j e2[j] pp[j,i] accumulate on the PE in fp32 PSUM;
  1/den via exp(-ln(den)) plus one Newton step; xT += yT * (1/den).

Host side (the sharding step): x -> xT, adj -> maskT fp16, outputs come back
transposed and are flipped on the host. All device DMA is contiguous (the
transposed-access DMA path measures ~5 GB/s on this setup, so layout changes
live on the host).
"""

import numpy as np

B, N, D, L = 8, 2048, 128, 3
NT = N // 128
NCH = N // 512


def _apply_tilefix():
    """This walrus build rejects >1 sync wait on an instruction; the stock
    Tile exit-drain carries several. Spread them across single-wait NOPs."""
    import concourse.tile as tile_mod
    from concourse import mybir

    def _patched_drain_and_barrier(self, tick_clock, wait_clock):
        from concourse.tile import ScopedClock

        drain_inst = self.nc.sync.drain()
        wait_clock.add_sem_waits(
            drain_inst.ins, ScopedClock({None: tick_clock.global_clock})
        )
        si = drain_inst.ins.sync_info
        if si is not None and len(si.on_wait) > 1:
            extra = list(si.on_wait[1:])
            del si.on_wait[1:]
            for w in extra:
                nop = self.nc.sync.nop()
                nop.ins.sync_info = mybir.SyncInfo(on_wait=[w], on_update=[])
        self.nc.all_engine_barrier()
        assert self.sems is not None
        popped = self.nc._tile_sem_poison_stack.pop()
        assert popped is self._sem_poison
        self.nc.clear_and_free_semaphores(list(self.sems.allocated().values()))
        self.nc.all_engine_barrier()

    tile_mod.TileContext._drain_and_barrier = _patched_drain_and_barrier


def _split_sync_waits(nc):
    """Hoist extra sync waits onto same-engine NOPs (walrus one-wait limit)."""
    from concourse import mybir

    n = 0
    for fn in nc.m.functions:
        for bb in fn.blocks:
            new_insts = []
            changed = False
            for inst in bb.instructions:
                si = inst.sync_info
                if si is not None and len(si.on_wait) > 1:
                    extra = list(si.on_wait[1:])
                    del si.on_wait[1:]
                    for w in extra:
                        nop = mybir.InstNoOp(name=f"waitsplit-{n}", ins=[], outs=[])
                        n += 1
                        nop.engine = inst.engine
                        nop.sync_info = mybir.SyncInfo(on_wait=[w], on_update=[])
                        new_insts.append(nop)
                    changed = True
                new_insts.append(inst)
            if changed:
                bb.instructions[:] = new_insts
    return n


def build_gat(reps=1, gps_mask_tiles=6):
    """Build the Bass program. reps>1 wraps the body in a For_i (timing)."""
    import contextlib

    import concourse.bass as bass
    import concourse.tile as tile
    from concourse import mybir

    f32 = mybir.dt.float32
    f16 = mybir.dt.float16
    A = mybir.AluOpType
    F = mybir.ActivationFunctionType

    nc = bass.Bass("TRN2", target_bir_lowering=False, debug=False, num_devices=8)

    xt_in = nc.dram_tensor("xt_in", [128, N], f32, kind="ExternalInput").ap()
    maskt = nc.dram_tensor("maskt", [N, N], f16, kind="ExternalInput").ap()
    wg = nc.dram_tensor("wg", [L, D, D], f32, kind="ExternalInput").ap()
    bg = nc.dram_tensor("bg", [L, D], f32, kind="ExternalInput").ap()
    aa = nc.dram_tensor("aa", [L, 2 * D], f32, kind="ExternalInput").ap()
    xt_out = nc.dram_tensor("xt_out", [128, N], f32, kind="ExternalOutput").ap()

    with tile.TileContext(nc) as tc:
        ctx = contextlib.ExitStack()
        with ctx:
            consts = ctx.enter_context(tc.tile_pool(name="consts", bufs=1))
            mask_pool = ctx.enter_context(tc.tile_pool(name="mask", bufs=1))
            xt_pool = ctx.enter_context(tc.tile_pool(name="xt", bufs=2))
            hT_pool = ctx.enter_context(tc.tile_pool(name="hT", bufs=1))
            hh_pool = ctx.enter_context(tc.tile_pool(name="hh", bufs=1))
            vec_pool = ctx.enter_context(tc.tile_pool(name="vec", bufs=1))
            att_pool = ctx.enter_context(tc.tile_pool(name="att", bufs=3))
            norm_pool = ctx.enter_context(tc.tile_pool(name="norm", bufs=1))

            ones_row = consts.tile([1, 128], f32)
            nc.vector.memset(ones_row, 1.0)
            ones128 = consts.tile([128, 128], f16)
            nc.vector.memset(ones128, 1.0)
            Ws, b_cols, a1cols, a2cols, b_rows = [], [], [], [], []
            for l in range(L):
                W = consts.tile([128, 128], f32, tag=f"W{l}")
                nc.gpsimd.dma_start(out=W[:], in_=wg[l])
                Ws.append(W)
                bc = consts.tile([128, 1], f32, tag=f"bc{l}")
                nc.gpsimd.dma_start(
                    out=bc[:], in_=bg[l].rearrange("(d one) -> d one", one=1)
                )
                b_cols.append(bc)
                a1c = consts.tile([128, 1], f32, tag=f"a1{l}")
                nc.gpsimd.dma_start(
                    out=a1c[:], in_=aa[l, 0:D].rearrange("(d one) -> d one", one=1)
                )
                a1cols.append(a1c)
                a2c = consts.tile([128, 1], f32, tag=f"a2{l}")
                nc.gpsimd.dma_start(
                    out=a2c[:],
                    in_=aa[l, D : 2 * D].rearrange("(d one) -> d one", one=1),
                )
                a2cols.append(a2c)
                br = consts.tile([1, 128], f32, tag=f"br{l}")
                nc.gpsimd.dma_start(
                    out=br[:], in_=bg[l].rearrange("(one d) -> one d", one=1)
                )
                b_rows.append(br)

            def body():
                mask_sb = mask_pool.tile([128, NT * N], f16)
                xT = xt_pool.tile([128, N], f32)
                nc.sync.dma_start(out=xT[:], in_=xt_in[:])
                for jt in range(NT):
                    nc.sync.dma_start(
                        out=mask_sb[:, jt * N : (jt + 1) * N],
                        in_=maskt[jt * 128 : (jt + 1) * 128, :],
                    )

                for l in range(L):
                    W = Ws[l]
                    prep_ctx = contextlib.ExitStack()
                    ps_big = prep_ctx.enter_context(
                        tc.tile_pool(name=f"ps_big{l}", bufs=1, space="PSUM")
                    )
                    ps_small = prep_ctx.enter_context(
                        tc.tile_pool(name=f"ps_small{l}", bufs=2, space="PSUM")
                    )
                    ps_col = prep_ctx.enter_context(
                        tc.tile_pool(name=f"ps_col{l}", bufs=1, space="PSUM")
                    )
                    # hT = relu(W.T @ xT + b)
                    hT_ps = ps_big.tile([128, N], f32, tag="big")
                    for c in range(NCH):
                        sl = slice(c * 512, (c + 1) * 512)
                        nc.tensor.matmul(hT_ps[:, sl], W[:], xT[:, sl])
                    hT = hT_pool.tile([128, N], f32)
                    nc.scalar.activation(
                        hT[:], hT_ps[:], F.Relu, bias=b_cols[l][:], scale=1.0
                    )
                    # s2 as columns, e2 = exp(0.2 s2)
                    scols_ps = ps_col.tile([128, NT], f32, tag="scols")
                    for jt in range(NT):
                        nc.tensor.matmul(
                            scols_ps[:, jt : jt + 1],
                            hT[:, jt * 128 : (jt + 1) * 128],
                            a2cols[l][:],
                        )
                    scols = vec_pool.tile([128, NT], f32, tag="scols_sb")
                    nc.scalar.activation(scols[:], scols_ps[:], F.Copy)
                    e2cols = vec_pool.tile([128, NT], f32, tag="e2_sb")
                    nc.scalar.activation(e2cols[:], scols_ps[:], F.Exp, scale=0.2)
                    # s1 row
                    s1row = vec_pool.tile([1, N], f32, tag="s1row")
                    for c in range(NCH):
                        sl = slice(c * 512, (c + 1) * 512)
                        s1_ps = ps_small.tile([1, 512], f32, tag="small")
                        nc.tensor.matmul(s1_ps[:], a1cols[l][:], hT[:, sl])
                        nc.scalar.activation(s1row[:, sl], s1_ps[:], F.Copy)
                    # s1 broadcast across partitions, fp16
                    s1bc_ps = ps_big.tile([128, N], f32, tag="big")
                    for c in range(NCH):
                        sl = slice(c * 512, (c + 1) * 512)
                        nc.tensor.matmul(s1bc_ps[:, sl], ones_row[:], s1row[:, sl])
                    s1bc = vec_pool.tile([128, N], f16, tag="s1bc")
                    nc.scalar.activation(s1bc[:], s1bc_ps[:], F.Copy)
                    # h'' = relu(h)*e2 (fp16) and E2MAT = ones*e2
                    hh = hh_pool.tile([128, NT * 128], f16, tag="hh")
                    em = hh_pool.tile([128, NT * 128], f16, tag="em")
                    for jt in range(NT):
                        sl = slice(jt * 128, (jt + 1) * 128)
                        hpp_ps = ps_small.tile([128, 128], f32, tag="small")
                        nc.tensor.matmul(
                            hpp_ps[:], xT[:, sl], W[:], start=True, stop=False
                        )
                        nc.tensor.matmul(
                            hpp_ps[:], ones_row[:], b_rows[l][:], start=False, stop=True
                        )
                        nc.vector.tensor_scalar(
                            hh[:, sl], hpp_ps[:], 0.0, e2cols[:, jt : jt + 1],
                            A.max, A.mult,
                        )
                        nc.vector.tensor_scalar(
                            em[:, sl], ones128[:], e2cols[:, jt : jt + 1], None, A.mult
                        )
                    prep_ctx.close()
                    # attention: pp = mask * exp(0.8 relu(s1bc + s2col))
                    attn_ctx = contextlib.ExitStack()
                    ps_y = attn_ctx.enter_context(
                        tc.tile_pool(name=f"ps_y{l}", bufs=1, space="PSUM")
                    )
                    ps_d = attn_ctx.enter_context(
                        tc.tile_pool(name=f"ps_d{l}", bufs=1, space="PSUM")
                    )
                    yT_ps = ps_y.tile([128, N], f32, tag="y")
                    den_ps = ps_d.tile([128, N], f32, tag="d")
                    for jt in range(NT):
                        a_t = att_pool.tile([128, N], f16, tag="a_t")
                        nc.vector.tensor_scalar(
                            a_t[:], s1bc[:], scols[:, jt : jt + 1], 0.0, A.add, A.max
                        )
                        q = att_pool.tile([128, N], f16, tag="q")
                        nc.scalar.activation(q[:], a_t[:], F.Exp, scale=0.8)
                        pp = att_pool.tile([128, N], f16, tag="pp")
                        # interleave the GPSIMD mask-muls across the j-loop so
                        # DVE/GPSIMD stay balanced throughout (-4us/iter vs
                        # front-loading, measured same-process)
                        eng = nc.gpsimd if jt in (1, 4, 6, 9, 12, 14) else nc.vector
                        eng.tensor_tensor(
                            pp[:], q[:], mask_sb[:, jt * N : (jt + 1) * N], A.mult
                        )
                        hsl = slice(jt * 128, (jt + 1) * 128)
                        for c in range(NCH):
                            sl = slice(c * 512, (c + 1) * 512)
                            nc.tensor.matmul(
                                yT_ps[:, sl], hh[:, hsl], pp[:, sl],
                                start=(jt == 0), stop=(jt == NT - 1),
                            )
                        for c in range(NCH):
                            sl = slice(c * 512, (c + 1) * 512)
                            nc.tensor.matmul(
                                den_ps[:, sl], em[:, hsl], pp[:, sl],
                                start=(jt == 0), stop=(jt == NT - 1),
                            )
                    # 1/den = exp(-ln den) + one Newton step
                    lnd = norm_pool.tile([128, N], f32, tag="nA")
                    nc.scalar.activation(lnd[:], den_ps[:], F.Ln)
                    r0 = norm_pool.tile([128, N], f32, tag="nB")
                    nc.scalar.activation(r0[:], lnd[:], F.Exp, scale=-1.0)
                    t1 = norm_pool.tile([128, N], f32, tag="nC")
                    nc.vector.tensor_tensor(t1[:], den_ps[:], r0[:], A.mult)
                    u = norm_pool.tile([128, N], f32, tag="nA")
                    nc.vector.tensor_scalar(u[:], t1[:], -1.0, 2.0, A.mult, A.add)
                    r1 = norm_pool.tile([128, N], f32, tag="nC")
                    nc.vector.tensor_tensor(r1[:], r0[:], u[:], A.mult)
                    # xT_new = xT + yT * r1
                    ytmp = norm_pool.tile([128, N], f32, tag="nB")
                    nc.vector.tensor_tensor(ytmp[:], yT_ps[:], r1[:], A.mult)
                    xT_new = xt_pool.tile([128, N], f32)
                    nc.vector.tensor_tensor(xT_new[:], ytmp[:], xT[:], A.add)
                    attn_ctx.close()
                    xT = xT_new

                nc.sync.dma_start(out=xt_out[:], in_=xT[:])

            if reps == 1:
                body()
            else:
                with tc.For_i(0, reps, 1):
                    body()

    return nc


def host_prep(x, adj, Wg, bg, attn_a):
    in_maps = []
    for b in range(B):
        in_maps.append(
            {
                "xt_in": np.ascontiguousarray(x[b].T).astype(np.float32),
                "maskt": np.ascontiguousarray(adj[b].T > 0).astype(np.float16),
                "wg": np.ascontiguousarray(Wg, np.float32),
                "bg": np.ascontiguousarray(bg, np.float32),
                "aa": np.ascontiguousarray(attn_a, np.float32),
            }
        )
    return in_maps


def host_post(results):
    return np.stack([results[b]["xt_out"].T for b in range(B)]).astype(np.float32)


def kernel(x, adj, Wg, bg, attn_a):
    x = np.asarray(x)
    adj = np.asarray(adj)
    Wg = np.asarray(Wg)
    bg = np.asarray(bg)
    attn_a = np.asarray(attn_a)

    _apply_tilefix()
    from concourse.bass_utils import run_bass_kernel_spmd

    nc = build_gat(reps=1)
    _split_sync_waits(nc)
    in_maps = host_prep(x, adj, Wg, bg, attn_a)
    res = run_bass_kernel_spmd(nc, in_maps, core_ids=list(range(B)))
    return host_post(res.results)


# revision 8
# speedup vs baseline: 4.3441x; 3.7171x over previous
"""TRN2 Bass kernel for nn_GAT (gnn_message_passing).

3-layer GAT stack: per layer h = relu(x@W+b); e = lrelu(s1[i]+s2[j]) masked by
adj; x += softmax_j(e) @ h.   B=8 graphs, N=2048 nodes, D=128 features.

Sharding: data-parallel over the batch dim - one graph per NeuronCore (8
cores), tiny per-layer weights replicated to every core.

This setup is HBM-DMA-bound (~43 GB/s/core effective with all 8 cores
pulling), so the adjacency mask - the only O(N^2) input - is shipped as
PACKED BITS: words[p,i] bit jt = adj[i, jt*128+p], one uint16 word per
(partition, node-i) = 512 KB/core instead of 8 MB fp16.  On device the 16
bit-planes are peeled to fp16 {0,1} with two tensor_scalar ops each
(AND-immediate, then *2^-jt; the DVE rejects bitwise+arith fused pairs).

Compute (per core, features on partitions, node index on the free axis):
  exp(lrelu(t)) with t = s1[i]+s2[j] factors as exp(0.2t)*max(exp(0.8t), 1),
  and exp(0.8t) is RANK-1, so after softmax cancellation of exp(0.2 s1):
    pp[j,i] = maskbit[j,i] * max(ubc[i]*w[j], e2[j]) ,
    ubc = exp(0.8 s1) broadcast,  w = exp(s2),  e2 = exp(0.2 s2)
  Per 128x2048 j-tile the NxN work is ONE DVE tensor_scalar (4x mode) plus
  ONE tensor_tensor multiply with the bit-plane - no N^2 ACT exp at all
  (exp runs only on the N-sized s1/s2 vectors).
  yT[d,i] = sum_j relu(h)[j,d] pp[j,i] and den[i] = sum_j pp[j,i] accumulate
  on the PE in fp32 PSUM (fp16 matmuls; den uses a constant all-ones
  stationary); 1/den = exp(-ln den) on ACT; xT += yT * (1/den).

Host side: x -> xT fp16, adj -> packed bit words, weights pre-cast fp16;
outputs return transposed fp16 and are flipped/cast on the host.
"""

import numpy as np

B, N, D, L = 8, 2048, 128, 3
NT = N // 128
NCH = N // 512


def _apply_tilefix():
    """This walrus build rejects >1 sync wait on an instruction; the stock
    Tile exit-drain carries several. Spread them across single-wait NOPs."""
    import concourse.tile as tile_mod
    from concourse import mybir

    def _patched_drain_and_barrier(self, tick_clock, wait_clock):
        from concourse.tile import ScopedClock

        drain_inst = self.nc.sync.drain()
        wait_clock.add_sem_waits(
            drain_inst.ins, ScopedClock({None: tick_clock.global_clock})
        )
        si = drain_inst.ins.sync_info
        if si is not None and len(si.on_wait) > 1:
            extra = list(si.on_wait[1:])
            del si.on_wait[1:]
            for w in extra:
                nop = self.nc.sync.nop()
                nop.ins.sync_info = mybir.SyncInfo(on_wait=[w], on_update=[])
        self.nc.all_engine_barrier()
        assert self.sems is not None
        popped = self.nc._tile_sem_poison_stack.pop()
        assert popped is self._sem_poison
        self.nc.clear_and_free_semaphores(list(self.sems.allocated().values()))
        self.nc.all_engine_barrier()

    tile_mod.TileContext._drain_and_barrier = _patched_drain_and_barrier


def _split_sync_waits(nc):
    """Hoist extra sync waits onto same-engine NOPs (walrus one-wait limit)."""
    from concourse import mybir

    n = 0
    for fn in nc.m.functions:
        for bb in fn.blocks:
            new_insts = []
            changed = False
            for inst in bb.instructions:
                si = inst.sync_info
                if si is not None and len(si.on_wait) > 1:
                    extra = list(si.on_wait[1:])
                    del si.on_wait[1:]
                    for w in extra:
                        nop = mybir.InstNoOp(name=f"waitsplit-{n}", ins=[], outs=[])
                        n += 1
                        nop.engine = inst.engine
                        nop.sync_info = mybir.SyncInfo(on_wait=[w], on_update=[])
                        new_insts.append(nop)
                    changed = True
                new_insts.append(inst)
            if changed:
                bb.instructions[:] = new_insts
    return n


def build_gat(reps=1):
    """Build the Bass program. reps>1 wraps the body in a For_i (timing)."""
    import contextlib

    import concourse.bass as bass
    import concourse.tile as tile
    from concourse import mybir

    f32 = mybir.dt.float32
    f16 = mybir.dt.float16
    u16 = mybir.dt.uint16
    A = mybir.AluOpType
    F = mybir.ActivationFunctionType

    nc = bass.Bass("TRN2", target_bir_lowering=False, debug=False, num_devices=8)

    xt_in = nc.dram_tensor("xt_in", [128, N], f16, kind="ExternalInput").ap()
    mbits = nc.dram_tensor("mbits", [128, N], u16, kind="ExternalInput").ap()
    wg16 = nc.dram_tensor("wg16", [L, D, D], f16, kind="ExternalInput").ap()
    bgc = nc.dram_tensor("bgc", [L, D], f32, kind="ExternalInput").ap()
    bgr16 = nc.dram_tensor("bgr16", [L, D], f16, kind="ExternalInput").ap()
    aa16 = nc.dram_tensor("aa16", [L, 2 * D], f16, kind="ExternalInput").ap()
    xt_out = nc.dram_tensor("xt_out", [128, N], f16, kind="ExternalOutput").ap()

    with tile.TileContext(nc) as tc:
        ctx = contextlib.ExitStack()
        with ctx:
            consts = ctx.enter_context(tc.tile_pool(name="consts", bufs=1))
            mask_pool = ctx.enter_context(tc.tile_pool(name="mask", bufs=1))
            words_pool = ctx.enter_context(tc.tile_pool(name="words", bufs=2))
            xt_pool = ctx.enter_context(tc.tile_pool(name="xt", bufs=2))
            hT_pool = ctx.enter_context(tc.tile_pool(name="hT", bufs=1))
            hh_pool = ctx.enter_context(tc.tile_pool(name="hh", bufs=1))
            ubc_pool = ctx.enter_context(tc.tile_pool(name="ubc", bufs=1))
            vec_pool = ctx.enter_context(tc.tile_pool(name="vec", bufs=1))
            att_pool = ctx.enter_context(tc.tile_pool(name="att", bufs=3))
            norm_pool = ctx.enter_context(tc.tile_pool(name="norm", bufs=1))

            ones_row = consts.tile([1, 128], f16)
            nc.vector.memset(ones_row, 1.0)
            ones128 = consts.tile([128, 128], f16)
            nc.vector.memset(ones128, 1.0)
            Ws, b_cols, a1cols, a2cols, b_rows = [], [], [], [], []
            for l in range(L):
                W = consts.tile([128, 128], f16, tag=f"W{l}")
                nc.gpsimd.dma_start(out=W[:], in_=wg16[l])
                Ws.append(W)
                bc = consts.tile([128, 1], f32, tag=f"bc{l}")
                nc.gpsimd.dma_start(
                    out=bc[:], in_=bgc[l].rearrange("(d one) -> d one", one=1)
                )
                b_cols.append(bc)
                a1c = consts.tile([128, 1], f16, tag=f"a1{l}")
                nc.gpsimd.dma_start(
                    out=a1c[:], in_=aa16[l, 0:D].rearrange("(d one) -> d one", one=1)
                )
                a1cols.append(a1c)
                a2c = consts.tile([128, 1], f16, tag=f"a2{l}")
                nc.gpsimd.dma_start(
                    out=a2c[:],
                    in_=aa16[l, D : 2 * D].rearrange("(d one) -> d one", one=1),
                )
                a2cols.append(a2c)
                br = consts.tile([1, 128], f16, tag=f"br{l}")
                nc.gpsimd.dma_start(
                    out=br[:], in_=bgr16[l].rearrange("(one d) -> one d", one=1)
                )
                b_rows.append(br)

            def body():
                words = words_pool.tile([128, N], u16)
                nc.sync.dma_start(out=words[:], in_=mbits[:])
                x16 = xt_pool.tile([128, N], f16)
                nc.sync.dma_start(out=x16[:], in_=xt_in[:])
                # peel the 16 bit-planes to fp16 {0,1}: and-imm, then *2^-jt
                tpl = mask_pool.tile([128, NT * N], f16)
                for jt in range(NT):
                    ta = words_pool.tile([128, N], u16, tag="ta")
                    nc.vector.tensor_scalar(
                        ta[:], words[:], 1 << jt, None, A.bitwise_and
                    )
                    nc.vector.tensor_scalar(
                        tpl[:, jt * N : (jt + 1) * N], ta[:],
                        float(2.0 ** -jt), None, A.mult,
                    )

                for l in range(L):
                    W = Ws[l]
                    prep_ctx = contextlib.ExitStack()
                    ps_big = prep_ctx.enter_context(
                        tc.tile_pool(name=f"ps_big{l}", bufs=1, space="PSUM")
                    )
                    ps_small = prep_ctx.enter_context(
                        tc.tile_pool(name=f"ps_small{l}", bufs=2, space="PSUM")
                    )
                    ps_col = prep_ctx.enter_context(
                        tc.tile_pool(name=f"ps_col{l}", bufs=1, space="PSUM")
                    )
                    # hT = relu(W.T @ xT + b)   [feat, node] fp16
                    hT_ps = ps_big.tile([128, N], f32, tag="big")
                    for c in range(NCH):
                        sl = slice(c * 512, (c + 1) * 512)
                        nc.tensor.matmul(hT_ps[:, sl], W[:], x16[:, sl])
                    hT = hT_pool.tile([128, N], f16)
                    nc.scalar.activation(
                        hT[:], hT_ps[:], F.Relu, bias=b_cols[l][:], scale=1.0
                    )
                    # s2 columns -> w' = exp(s2)*2^(4-jt), e2' = exp(.2 s2)*2^(4-jt)
                    scols_ps = ps_col.tile([128, NT], f32, tag="scols")
                    for jt in range(NT):
                        nc.tensor.matmul(
                            scols_ps[:, jt : jt + 1],
                            hT[:, jt * 128 : (jt + 1) * 128],
                            a2cols[l][:],
                        )
                    wS = vec_pool.tile([128, NT], f32, tag="wS_sb")
                    nc.scalar.activation(wS[:], scols_ps[:], F.Exp, scale=1.0)
                    e2S = vec_pool.tile([128, NT], f32, tag="e2S_sb")
                    nc.scalar.activation(e2S[:], scols_ps[:], F.Exp, scale=0.2)
                    # s1 row (fp16, copied out of PSUM per 512-chunk on DVE)
                    s1row = vec_pool.tile([1, N], f16, tag="s1row")
                    for c in range(NCH):
                        sl = slice(c * 512, (c + 1) * 512)
                        s1_ps = ps_small.tile([1, 512], f32, tag="small")
                        nc.tensor.matmul(s1_ps[:], a1cols[l][:], hT[:, sl])
                        nc.vector.tensor_scalar(
                            s1row[:, sl], s1_ps[:], 0.0, None, A.add
                        )
                    # ubc = exp(0.8*s1) broadcast to 128 partitions (exp fused
                    # into the PSUM->SBUF copy on ACT)
                    ubc_ps = ps_big.tile([128, N], f32, tag="big")
                    for c in range(NCH):
                        sl = slice(c * 512, (c + 1) * 512)
                        nc.tensor.matmul(ubc_ps[:, sl], ones_row[:], s1row[:, sl])
                    ubc = ubc_pool.tile([128, N], f16)
                    nc.scalar.activation(ubc[:], ubc_ps[:], F.Exp, scale=0.8)
                    # hh = relu(h) node-major, one big PSUM strip, chunked
                    # relu-copies so attention can start on early tiles
                    hpp_ps = ps_big.tile([128, N], f32, tag="big")
                    hh = hh_pool.tile([128, NT * 128], f16, tag="hh")
                    for jt in range(NT):
                        sl = slice(jt * 128, (jt + 1) * 128)
                        nc.tensor.matmul(
                            hpp_ps[:, sl], x16[:, sl], W[:], start=True, stop=False
                        )
                        nc.tensor.matmul(
                            hpp_ps[:, sl], ones_row[:], b_rows[l][:],
                            start=False, stop=True,
                        )
                    for c in range(NCH):
                        sl = slice(c * 512, (c + 1) * 512)
                        nc.vector.tensor_scalar(
                            hh[:, sl], hpp_ps[:, sl], 0.0, None, A.max
                        )
                    prep_ctx.close()
                    # attention: pp = bitplane * max(ubc*w'_j, e2'_j)
                    attn_ctx = contextlib.ExitStack()
                    ps_y = attn_ctx.enter_context(
                        tc.tile_pool(name=f"ps_y{l}", bufs=1, space="PSUM")
                    )
                    ps_d = attn_ctx.enter_context(
                        tc.tile_pool(name=f"ps_d{l}", bufs=1, space="PSUM")
                    )
                    yT_ps = ps_y.tile([128, N], f32, tag="y")
                    den_ps = ps_d.tile([128, N], f32, tag="d")
                    for jt in range(NT):
                        q = att_pool.tile([128, N], f16, tag="q")
                        nc.vector.tensor_scalar(
                            q[:], ubc[:], wS[:, jt : jt + 1],
                            e2S[:, jt : jt + 1], A.mult, A.max,
                        )
                        pp = att_pool.tile([128, N], f16, tag="pp")
                        nc.vector.tensor_tensor(
                            pp[:], q[:], tpl[:, jt * N : (jt + 1) * N], A.mult
                        )
                        hsl = slice(jt * 128, (jt + 1) * 128)
                        for c in range(NCH):
                            sl = slice(c * 512, (c + 1) * 512)
                            nc.tensor.matmul(
                                yT_ps[:, sl], hh[:, hsl], pp[:, sl],
                                start=(jt == 0), stop=(jt == NT - 1),
                            )
                        for c in range(NCH):
                            sl = slice(c * 512, (c + 1) * 512)
                            nc.tensor.matmul(
                                den_ps[:, sl], ones128[:], pp[:, sl],
                                start=(jt == 0), stop=(jt == NT - 1),
                            )
                    # r = 1/den = exp(-ln den); xT_new = xT + yT * r
                    lnd = norm_pool.tile([128, N], f32, tag="nA")
                    nc.scalar.activation(lnd[:], den_ps[:], F.Ln)
                    r16 = norm_pool.tile([128, N], f16, tag="nB")
                    nc.scalar.activation(r16[:], lnd[:], F.Exp, scale=-1.0)
                    y16 = norm_pool.tile([128, N], f16, tag="nC")
                    nc.vector.tensor_scalar(y16[:], yT_ps[:], 0.0, None, A.add)
                    ytmp = norm_pool.tile([128, N], f16, tag="nD")
                    nc.vector.tensor_tensor(ytmp[:], y16[:], r16[:], A.mult)
                    attn_ctx.close()
                    x_new = xt_pool.tile([128, N], f16)
                    nc.vector.tensor_tensor(x_new[:], ytmp[:], x16[:], A.add)
                    x16 = x_new

                nc.sync.dma_start(out=xt_out[:], in_=x16[:])

            if reps == 1:
                body()
            else:
                with tc.For_i(0, reps, 1):
                    body()

    return nc


def host_prep(x, adj, Wg, bg, attn_a):
    in_maps = []
    for b in range(B):
        adjT = np.ascontiguousarray(adj[b].T) > 0  # [j, i]
        m3 = adjT.reshape(NT, 128, N).astype(np.uint16)
        words = np.zeros((128, N), np.uint16)
        for jt in range(NT):
            words |= m3[jt] << jt
        in_maps.append(
            {
                "xt_in": np.ascontiguousarray(x[b].T).astype(np.float16),
                "mbits": words,
                "wg16": np.ascontiguousarray(Wg).astype(np.float16),
                "bgc": np.ascontiguousarray(bg, np.float32),
                "bgr16": np.ascontiguousarray(bg).astype(np.float16),
                "aa16": np.ascontiguousarray(attn_a).astype(np.float16),
            }
        )
    return in_maps


def host_post(results):
    return np.stack([results[b]["xt_out"].T for b in range(B)]).astype(np.float32)


def kernel(x, adj, Wg, bg, attn_a):
    x = np.asarray(x)
    adj = np.asarray(adj)
    Wg = np.asarray(Wg)
    bg = np.asarray(bg)
    attn_a = np.asarray(attn_a)

    _apply_tilefix()
    from concourse.bass_utils import run_bass_kernel_spmd

    nc = build_gat(reps=1)
    _split_sync_waits(nc)
    in_maps = host_prep(x, adj, Wg, bg, attn_a)
    res = run_bass_kernel_spmd(nc, in_maps, core_ids=list(range(B)))
    return host_post(res.results)


# revision 11
# speedup vs baseline: 6.1000x; 1.4042x over previous
"""TRN2 Bass kernel for nn_GAT (gnn_message_passing).

3-layer GAT stack: per layer h = relu(x@W+b); e = lrelu(s1[i]+s2[j]) masked by
adj; x += softmax_j(e) @ h.   B=8 graphs, N=2048 nodes, D=128 features.

Sharding: data-parallel over the batch dim - one graph per NeuronCore (8
cores), tiny per-layer weights replicated to every core.

This setup is HBM-DMA-bound (~43 GB/s/core effective with all 8 cores
pulling), so the adjacency mask - the only O(N^2) input - is shipped as
PACKED BITS: words[p,i] bit jt = adj[i, jt*128+p], one uint16 word per
(partition, node-i) = 512 KB/core instead of 8 MB fp16.  On device the 16
bit-planes are peeled to fp16 {0,1} with two tensor_scalar ops each
(AND-immediate, then *2^-jt; the DVE rejects bitwise+arith fused pairs).

Compute (per core, features on partitions, node index on the free axis):
  exp(lrelu(t)) with t = s1[i]+s2[j] factors as exp(0.2t)*max(exp(0.8t), 1),
  and exp(0.8t) is RANK-1, so after softmax cancellation of exp(0.2 s1):
    pp[j,i] = maskbit[j,i] * max(ubc[i]*w[j], e2[j]) ,
    ubc = exp(0.8 s1) broadcast,  w = exp(s2),  e2 = exp(0.2 s2)
  Per 128x2048 j-tile the NxN work is ONE DVE tensor_scalar (4x mode) plus
  ONE tensor_tensor multiply with the bit-plane - no N^2 ACT exp at all
  (exp runs only on the N-sized s1/s2 vectors).
  yT[d,i] = sum_j relu(h)[j,d] pp[j,i] and den[i] = sum_j pp[j,i] accumulate
  on the PE in fp32 PSUM (fp16 matmuls; den uses a constant all-ones
  stationary); 1/den = exp(-ln den) on ACT; xT += yT * (1/den).

Host side: x -> xT fp16, adj -> packed bit words, weights pre-cast fp16;
outputs return transposed fp16 and are flipped/cast on the host.
"""

import numpy as np

B, N, D, L = 8, 2048, 128, 3
NT = N // 128
NCH = N // 512


def _apply_tilefix():
    """This walrus build rejects >1 sync wait on an instruction; the stock
    Tile exit-drain carries several. Spread them across single-wait NOPs."""
    import concourse.tile as tile_mod
    from concourse import mybir

    def _patched_drain_and_barrier(self, tick_clock, wait_clock):
        from concourse.tile import ScopedClock

        drain_inst = self.nc.sync.drain()
        wait_clock.add_sem_waits(
            drain_inst.ins, ScopedClock({None: tick_clock.global_clock})
        )
        si = drain_inst.ins.sync_info
        if si is not None and len(si.on_wait) > 1:
            extra = list(si.on_wait[1:])
            del si.on_wait[1:]
            for w in extra:
                nop = self.nc.sync.nop()
                nop.ins.sync_info = mybir.SyncInfo(on_wait=[w], on_update=[])
        self.nc.all_engine_barrier()
        assert self.sems is not None
        popped = self.nc._tile_sem_poison_stack.pop()
        assert popped is self._sem_poison
        self.nc.clear_and_free_semaphores(list(self.sems.allocated().values()))
        self.nc.all_engine_barrier()

    tile_mod.TileContext._drain_and_barrier = _patched_drain_and_barrier


def _split_sync_waits(nc):
    """Hoist extra sync waits onto same-engine NOPs (walrus one-wait limit)."""
    from concourse import mybir

    n = 0
    for fn in nc.m.functions:
        for bb in fn.blocks:
            new_insts = []
            changed = False
            for inst in bb.instructions:
                si = inst.sync_info
                if si is not None and len(si.on_wait) > 1:
                    extra = list(si.on_wait[1:])
                    del si.on_wait[1:]
                    for w in extra:
                        nop = mybir.InstNoOp(name=f"waitsplit-{n}", ins=[], outs=[])
                        n += 1
                        nop.engine = inst.engine
                        nop.sync_info = mybir.SyncInfo(on_wait=[w], on_update=[])
                        new_insts.append(nop)
                    changed = True
                new_insts.append(inst)
            if changed:
                bb.instructions[:] = new_insts
    return n


def build_gat(reps=1):
    """Build the Bass program. reps>1 wraps the body in a For_i (timing)."""
    import contextlib

    import concourse.bass as bass
    import concourse.tile as tile
    from concourse import mybir

    f32 = mybir.dt.float32
    f16 = mybir.dt.float16
    u16 = mybir.dt.uint16
    A = mybir.AluOpType
    F = mybir.ActivationFunctionType

    nc = bass.Bass("TRN2", target_bir_lowering=False, debug=False, num_devices=8)

    xt_in = nc.dram_tensor("xt_in", [128, N], f16, kind="ExternalInput").ap()
    mbits = nc.dram_tensor("mbits", [128, N], u16, kind="ExternalInput").ap()
    wg16 = nc.dram_tensor("wg16", [L, D, D], f16, kind="ExternalInput").ap()
    bgc = nc.dram_tensor("bgc", [L, D], f32, kind="ExternalInput").ap()
    bgr16 = nc.dram_tensor("bgr16", [L, D], f16, kind="ExternalInput").ap()
    aa16 = nc.dram_tensor("aa16", [L, 2 * D], f16, kind="ExternalInput").ap()
    xt_out = nc.dram_tensor("xt_out", [128, N], f16, kind="ExternalOutput").ap()

    with tile.TileContext(nc) as tc:
        ctx = contextlib.ExitStack()
        with ctx:
            consts = ctx.enter_context(tc.tile_pool(name="consts", bufs=1))
            mask_pool = ctx.enter_context(tc.tile_pool(name="mask", bufs=1))
            words_pool = ctx.enter_context(tc.tile_pool(name="words", bufs=2))
            xt_pool = ctx.enter_context(tc.tile_pool(name="xt", bufs=2))
            hT_pool = ctx.enter_context(tc.tile_pool(name="hT", bufs=1))
            hh_pool = ctx.enter_context(tc.tile_pool(name="hh", bufs=1))
            ubc_pool = ctx.enter_context(tc.tile_pool(name="ubc", bufs=1))
            vec_pool = ctx.enter_context(tc.tile_pool(name="vec", bufs=1))
            att_pool = ctx.enter_context(tc.tile_pool(name="att", bufs=4))
            norm_pool = ctx.enter_context(tc.tile_pool(name="norm", bufs=1))

            ones_row = consts.tile([1, 128], f16)
            nc.vector.memset(ones_row, 1.0)
            ones128 = consts.tile([128, 128], f16)
            nc.vector.memset(ones128, 1.0)
            Ws, b_cols, a1cols, a2cols, b_rows = [], [], [], [], []
            for l in range(L):
                W = consts.tile([128, 128], f16, tag=f"W{l}")
                nc.gpsimd.dma_start(out=W[:], in_=wg16[l])
                Ws.append(W)
                bc = consts.tile([128, 1], f32, tag=f"bc{l}")
                nc.gpsimd.dma_start(
                    out=bc[:], in_=bgc[l].rearrange("(d one) -> d one", one=1)
                )
                b_cols.append(bc)
                a1c = consts.tile([128, 1], f16, tag=f"a1{l}")
                nc.gpsimd.dma_start(
                    out=a1c[:], in_=aa16[l, 0:D].rearrange("(d one) -> d one", one=1)
                )
                a1cols.append(a1c)
                a2c = consts.tile([128, 1], f16, tag=f"a2{l}")
                nc.gpsimd.dma_start(
                    out=a2c[:],
                    in_=aa16[l, D : 2 * D].rearrange("(d one) -> d one", one=1),
                )
                a2cols.append(a2c)
                br = consts.tile([1, 128], f16, tag=f"br{l}")
                nc.gpsimd.dma_start(
                    out=br[:], in_=bgr16[l].rearrange("(one d) -> one d", one=1)
                )
                b_rows.append(br)

            def body():
                words = words_pool.tile([128, N], u16)
                nc.sync.dma_start(out=words[:], in_=mbits[:])
                x16 = xt_pool.tile([128, N], f16)
                nc.sync.dma_start(out=x16[:], in_=xt_in[:])
                # peel the 16 bit-planes to fp16 {0,1}: and-imm, then *2^-jt
                tpl = mask_pool.tile([128, NT * N], f16)
                for jt in range(NT):
                    ta = words_pool.tile([128, N], u16, tag="ta")
                    nc.vector.tensor_scalar(
                        ta[:], words[:], 1 << jt, None, A.bitwise_and
                    )
                    nc.vector.tensor_scalar(
                        tpl[:, jt * N : (jt + 1) * N], ta[:],
                        float(2.0 ** -jt), None, A.mult,
                    )

                for l in range(L):
                    W = Ws[l]
                    prep_ctx = contextlib.ExitStack()
                    ps_big = prep_ctx.enter_context(
                        tc.tile_pool(name=f"ps_big{l}", bufs=1, space="PSUM")
                    )
                    ps_hpp = prep_ctx.enter_context(
                        tc.tile_pool(name=f"ps_hpp{l}", bufs=2, space="PSUM")
                    )
                    ps_col = prep_ctx.enter_context(
                        tc.tile_pool(name=f"ps_col{l}", bufs=1, space="PSUM")
                    )
                    # hT = relu(W.T @ xT + b)   [feat, node] fp16, chunked so
                    # downstream s-vector matmuls start early
                    hT_ps = ps_big.tile([128, N], f32, tag="big")
                    hT = hT_pool.tile([128, N], f16)
                    for c in range(NCH):
                        sl = slice(c * 512, (c + 1) * 512)
                        nc.tensor.matmul(hT_ps[:, sl], W[:], x16[:, sl])
                    for c in range(NCH):
                        sl = slice(c * 512, (c + 1) * 512)
                        nc.scalar.activation(
                            hT[:, sl], hT_ps[:, sl], F.Relu,
                            bias=b_cols[l][:], scale=1.0,
                        )
                    # s2 columns -> w = exp(s2), e2 = exp(0.2 s2)
                    scols_ps = ps_col.tile([128, NT], f32, tag="scols")
                    for jt in range(NT):
                        nc.tensor.matmul(
                            scols_ps[:, jt : jt + 1],
                            hT[:, jt * 128 : (jt + 1) * 128],
                            a2cols[l][:],
                        )
                    wS = vec_pool.tile([128, NT], f32, tag="wS_sb")
                    nc.scalar.activation(wS[:], scols_ps[:], F.Exp, scale=1.0)
                    e2S = vec_pool.tile([128, NT], f32, tag="e2S_sb")
                    nc.scalar.activation(e2S[:], scols_ps[:], F.Exp, scale=0.2)
                    # s1 row (fp16; PSUM strip reuses the big pool buffer)
                    s1_ps = ps_big.tile([1, N], f32, tag="big")
                    s1row = vec_pool.tile([1, N], f16, tag="s1row")
                    for c in range(NCH):
                        sl = slice(c * 512, (c + 1) * 512)
                        nc.tensor.matmul(s1_ps[:, sl], a1cols[l][:], hT[:, sl])
                        nc.vector.tensor_scalar(
                            s1row[:, sl], s1_ps[:, sl], 0.0, None, A.add
                        )
                    # ubc = exp(0.8*s1) broadcast to 128 partitions (exp fused
                    # into the PSUM->SBUF copy on ACT)
                    ubc_ps = ps_big.tile([128, N], f32, tag="big")
                    for c in range(NCH):
                        sl = slice(c * 512, (c + 1) * 512)
                        nc.tensor.matmul(ubc_ps[:, sl], ones_row[:], s1row[:, sl])
                    ubc = ubc_pool.tile([128, N], f16)
                    nc.scalar.activation(ubc[:], ubc_ps[:], F.Exp, scale=0.8)
                    # hh = relu(h) node-major in 512-wide PSUM chunks (own
                    # 2-bank pool: overlaps the ubc ACT copy on the PE)
                    hh = hh_pool.tile([128, NT * 128], f16, tag="hh")
                    for c in range(NCH):
                        sl = slice(c * 512, (c + 1) * 512)
                        hpp_ps = ps_hpp.tile([128, 512], f32, tag="hpp")
                        for k in range(4):
                            jsl = slice(c * 512 + k * 128, c * 512 + (k + 1) * 128)
                            psl = slice(k * 128, (k + 1) * 128)
                            nc.tensor.matmul(
                                hpp_ps[:, psl], x16[:, jsl], W[:],
                                start=True, stop=False,
                            )
                            nc.tensor.matmul(
                                hpp_ps[:, psl], ones_row[:], b_rows[l][:],
                                start=False, stop=True,
                            )
                        nc.vector.tensor_scalar(
                            hh[:, sl], hpp_ps[:], 0.0, None, A.max
                        )
                    prep_ctx.close()
                    # attention: pp = bitplane * max(ubc*w'_j, e2'_j)
                    attn_ctx = contextlib.ExitStack()
                    ps_y = attn_ctx.enter_context(
                        tc.tile_pool(name=f"ps_y{l}", bufs=1, space="PSUM")
                    )
                    ps_d = attn_ctx.enter_context(
                        tc.tile_pool(name=f"ps_d{l}", bufs=1, space="PSUM")
                    )
                    yT_ps = ps_y.tile([128, N], f32, tag="y")
                    den_ps = ps_d.tile([128, N], f32, tag="d")
                    for jt in range(NT):
                        q = att_pool.tile([128, N], f16, tag="q")
                        nc.vector.tensor_scalar(
                            q[:], ubc[:], wS[:, jt : jt + 1],
                            e2S[:, jt : jt + 1], A.mult, A.max,
                        )
                        pp = att_pool.tile([128, N], f16, tag="pp")
                        nc.vector.tensor_tensor(
                            pp[:], q[:], tpl[:, jt * N : (jt + 1) * N], A.mult
                        )
                        hsl = slice(jt * 128, (jt + 1) * 128)
                        for c in range(NCH):
                            sl = slice(c * 512, (c + 1) * 512)
                            nc.tensor.matmul(
                                yT_ps[:, sl], hh[:, hsl], pp[:, sl],
                                start=(jt == 0), stop=(jt == NT - 1),
                            )
                        for c in range(NCH):
                            sl = slice(c * 512, (c + 1) * 512)
                            nc.tensor.matmul(
                                den_ps[:, sl], ones128[:], pp[:, sl],
                                start=(jt == 0), stop=(jt == NT - 1),
                            )
                    # r = 1/den = exp(-ln den); xT_new = xT + yT * r.
                    # All in 512-chunks so the next layer's hT matmul starts
                    # while later chunks are still normalizing.
                    lnd = norm_pool.tile([128, N], f32, tag="nA")
                    r16 = norm_pool.tile([128, N], f16, tag="nB")
                    y16 = norm_pool.tile([128, N], f16, tag="nC")
                    ytmp = norm_pool.tile([128, N], f16, tag="nD")
                    x_new = xt_pool.tile([128, N], f16)
                    for c in range(NCH):
                        sl = slice(c * 512, (c + 1) * 512)
                        nc.scalar.activation(lnd[:, sl], den_ps[:, sl], F.Ln)
                        nc.scalar.activation(
                            r16[:, sl], lnd[:, sl], F.Exp, scale=-1.0
                        )
                        nc.vector.tensor_scalar(
                            y16[:, sl], yT_ps[:, sl], 0.0, None, A.add
                        )
                        nc.vector.tensor_tensor(
                            ytmp[:, sl], y16[:, sl], r16[:, sl], A.mult
                        )
                        nc.vector.tensor_tensor(
                            x_new[:, sl], ytmp[:, sl], x16[:, sl], A.add
                        )
                    attn_ctx.close()
                    x16 = x_new

                nc.sync.dma_start(out=xt_out[:], in_=x16[:])

            if reps == 1:
                body()
            else:
                with tc.For_i(0, reps, 1):
                    body()

    return nc


def host_prep(x, adj, Wg, bg, attn_a):
    in_maps = []
    for b in range(B):
        adjT = np.ascontiguousarray(adj[b].T) > 0  # [j, i]
        m3 = adjT.reshape(NT, 128, N).astype(np.uint16)
        words = np.zeros((128, N), np.uint16)
        for jt in range(NT):
            words |= m3[jt] << jt
        in_maps.append(
            {
                "xt_in": np.ascontiguousarray(x[b].T).astype(np.float16),
                "mbits": words,
                "wg16": np.ascontiguousarray(Wg).astype(np.float16),
                "bgc": np.ascontiguousarray(bg, np.float32),
                "bgr16": np.ascontiguousarray(bg).astype(np.float16),
                "aa16": np.ascontiguousarray(attn_a).astype(np.float16),
            }
        )
    return in_maps


def host_post(results):
    return np.stack([results[b]["xt_out"].T for b in range(B)]).astype(np.float32)


def kernel(x, adj, Wg, bg, attn_a):
    x = np.asarray(x)
    adj = np.asarray(adj)
    Wg = np.asarray(Wg)
    bg = np.asarray(bg)
    attn_a = np.asarray(attn_a)

    _apply_tilefix()
    from concourse.bass_utils import run_bass_kernel_spmd

    nc = build_gat(reps=1)
    _split_sync_waits(nc)
    in_maps = host_prep(x, adj, Wg, bg, attn_a)
    res = run_bass_kernel_spmd(nc, in_maps, core_ids=list(range(B)))
    return host_post(res.results)
